# revision 113
# baseline (speedup 1.0000x reference)
"""Trainium2 Bass kernel for nn_DelayedSelfAttention (B=4, T=1024, C=1024, H=16).

Sharding: 8 cores = 4 batches x 2 sequence-halves.  Core c handles batch
c//2 and query rows [r*T, (r+1)*T) of the concatenated [2T] sequence
(r = c%2).  Each core computes K/V for the full 2T sequence (duplicated
kv-projection -- cheaper than any collective on this fabric), attention
for its T query rows over all 16 heads, and the output projection for
its rows.  Role asymmetry (mask values, q/proj LoRA) is pushed into
per-core input data so a single SPMD program serves all cores.

v5 (329.7us -> 270.4us on the TimelineSim cost model):
 - Q-side and proj-side LoRA folded into the weights on the host
   (a core's query rows are single-role, so W_eff = W + role*(a/r)*A@B
   is exact); only the K/V LoRA stays on-device (row-dependent).
 - QKV projections as compensated fp8e4m3 DoubleRow matmuls
   (x8W8 + xloW8 + x8Wlo, weights prescaled by 64).
 - AV flipped q-major: the probability tile is the 128-wide stationary,
   the 65-col V tile (ones column appended for the softmax denominator)
   is the moving operand -- 65 vs nq cycles per (tile, qt, head) pair.
   PSUM accumulation uses ONE start/stop per bank (start marks the
   whole 2KB zero-region pending-zero, so per-qt starts would wipe
   sibling accumulations).  AV matmuls for a whole pair are deferred
   until after all its score/exp emissions (pt pool holds every tile)
   so exp latency never stalls the PE.
 - Normalization is a per-partition scalar mul; the [q,(hi d)] ->
   [(hi d),q] transpose rides the DMA xbar as ONE blocked 4x(128x128)
   DmaTransposeAnt per pair, so the PE never touches softmax
   bookkeeping.
 - K chunk-pairs and V head-halves are computed lazily inside the
   attention windows of the first pair that needs them; qb0-p0 MUST
   process its e1 tiles first (its bracket emits the s2-block K/V that
   the e2 tiles' scores read).
 - SBUF zones are placed by DMA urgency: wk / wv(vc0) / x-s0-2 take
   fresh space so their loads stream during Q compute; masks, la/lb,
   x-s3 and wv(vc1) reuse the freed Q zones (their DMAs legitimately
   wait out the last Q matmul); wproj later takes over the x-s0-2 slot
   via tile-tag reuse.  Q-phase operands arrive in fine first-use-order
   slices (xq by s/hi-lo quarter, wq by 128-col chunk).
"""

import contextlib
import sys

for _p in ("/opt/trn_rl_repo", "/root/.axon_site/_ro/trn_rl_repo"):
    if _p not in sys.path:
        sys.path.insert(0, _p)

import ml_dtypes
import numpy as np

import concourse.bass as bass
import concourse.mybir as mybir
import concourse.tile as tile_mod
from concourse.bass_utils import run_bass_kernel_spmd
from concourse.tile import TileContext
from concourse.vector_clock import ScopedClock

# ---------------------------------------------------------------------------
# Workaround: this walrus build supports a single semaphore wait per
# instruction.  Split multi-wait instructions into same-engine NoOps each
# carrying one wait (identical sequencer semantics).
# ---------------------------------------------------------------------------
_ws_counter = [0]


def _fresh_name():
    _ws_counter[0] += 1
    return f"I-waitsplit-{_ws_counter[0]}"


def _split_inst_waits(inst):
    si = inst.sync_info
    if si is None:
        return []
    waits = list(si.on_wait or [])
    if len(waits) <= 1:
        return []
    nops = []
    for w in waits[:-1]:
        nop = mybir.InstNoOp(name=_fresh_name())
        nop.engine = inst.engine
        nop.sync_info = mybir.SyncInfo(on_wait=[w], on_update=[])
        nops.append(nop)
    inst.sync_info = mybir.SyncInfo(
        on_wait=[waits[-1]], on_update=list(si.on_update or [])
    )
    return nops


_orig_lower = tile_mod.TileContext._lower_ordered_insts


def _patched_lower(self, ordered):
    for bb_name in list(ordered.keys()):
        new = []
        for inst in ordered[bb_name]:
            new.extend(_split_inst_waits(inst))
            new.append(inst)
        ordered[bb_name] = new
    return _orig_lower(self, ordered)


def _patched_drain_and_barrier(self, tick_clock, wait_clock):
    nc = self.nc
    drain_inst = nc.sync.drain()
    wait_clock.add_sem_waits(
        drain_inst.ins, ScopedClock({None: tick_clock.global_clock})
    )
    nops = _split_inst_waits(drain_inst.ins)
    if nops:
        first_wait = drain_inst.ins.sync_info
        drain_inst.ins.sync_info = mybir.SyncInfo(on_wait=[], on_update=[])
        for nop in nops:
            n2 = nc.sync.nop(nofuse=True)
            n2.ins.sync_info = nop.sync_info
        d2 = nc.sync.drain()
        d2.ins.sync_info = first_wait

    nc.all_engine_barrier()
    assert self.sems is not None
    popped = nc._tile_sem_poison_stack.pop()
    assert popped is self._sem_poison
    nc.clear_and_free_semaphores(list(self.sems.allocated().values()))
    nc.all_engine_barrier()


def _apply_tile_patch():
    if tile_mod.TileContext._lower_ordered_insts is not _patched_lower:
        tile_mod.TileContext._lower_ordered_insts = _patched_lower
        tile_mod.TileContext._drain_and_barrier = _patched_drain_and_barrier


# ---------------------------------------------------------------------------
# Problem constants (hardcoded per the task contract).
# ---------------------------------------------------------------------------
B, T, C, H = 4, 1024, 1024, 16
D = C // H  # 64
SEQ = 2 * T
LOOKAHEAD, OVERLAP = 64, 64
RANK, ALPHA = 8, 16.0
RPAD = 16  # lora-A stationary padded (dual-fp8 ldweights needs width >= 16)
LSCALE = ALPHA / RANK  # 2.0
QSCALE = 1.0 / np.sqrt(D)  # 1/8
WSC = 64.0  # fp8 weight prescale (cleared by 1/WSC at staging)
NCH = C // 128  # 8 c-chunks
NCP = NCH // 2  # 4 c-chunk-pairs (DoubleRow)
NQT = T // 128  # 8 q-subtiles per core
F32 = mybir.dt.float32
F32R = mybir.dt.float32r
BF16 = mybir.dt.bfloat16
F8E4 = mybir.dt.float8e4
FP8NP = ml_dtypes.float8_e4m3fn
DR = mybir.MatmulPerfMode.DoubleRow


# Trace-time tiling structure, shared by host (mask packing) and device.
def _ktiles_for_qblock(qb):
    """k-tiles (region, j) touched by q-subtiles [4qb, 4qb+4)."""
    qts = range(4 * qb, 4 * qb + 4)
    e1 = sorted({j for qt in qts for j in (qt - 1, qt, qt + 1) if 0 <= j < NQT})
    e2 = sorted({j for qt in qts for j in range(qt + 1)})
    return [("e1", j) for j in e1] + [("e2", j) for j in e2]


def _active_qts(region, j, qb):
    if region == "e1":
        qts = [qt for qt in range(4 * qb, 4 * qb + 4) if j in (qt - 1, qt, qt + 1)]
    else:
        qts = [qt for qt in range(4 * qb, 4 * qb + 4) if j <= qt]
    assert qts == list(range(qts[0], qts[-1] + 1))
    return qts


def _mask_tiles():
    out = []
    for qt in range(NQT):
        for j in (qt - 1, qt, qt + 1):
            if 0 <= j < NQT:
                out.append(("e1", j, qt))
        for j in (qt - 1, qt):
            if j >= 0:
                out.append(("e2", j, qt))
    return out


MASK_TILES = _mask_tiles()  # 37 tiles
MASK_IDX = {k: i for i, k in enumerate(MASK_TILES)}
NMASK = len(MASK_TILES)


# ---------------------------------------------------------------------------
# Device program
# ---------------------------------------------------------------------------
def _build_program():
    _apply_tile_patch()
    nc = bass.Bass("TRN2", target_bir_lowering=False, debug=False, num_devices=8)

    def din(name, shape, dt=F32R):
        return nc.dram_tensor(name, list(shape), dt, kind="ExternalInput").ap()

    # hi/lo fp8 pairs packed on one axis of the *_p tensors (one DMA each).
    x_p = din("x_p", (128, 2, NCP, 2, SEQ), dt=F8E4)
    # Q-phase operands arrive in fine slices in first-use order: xq by
    # (s, hi/lo) quarter, wq by 128-column chunk (contiguous 2KB runs),
    # so the first Q matmul unblocks ~3.5 us in.
    xq_p = din("xq_p", (2, 2, 128, NCP, 2, 512), dt=F8E4)
    wq_p = din("wq_p", (8, 128, 2, NCP, 2, 128), dt=F8E4)
    wk_p = din("wk_p", (128, 2, NCP, 2, C), dt=F8E4)
    wva_p = din("wva_p", (128, 2, NCP, 2, 512), dt=F8E4)
    wvb_p = din("wvb_p", (128, 2, NCP, 2, 512), dt=F8E4)
    la_p = din("la_p", (128, 2, NCP, 2, RPAD), dt=F8E4)
    lb_k = din("lb_k", (RANK, C), dt=BF16)   # k-lora B, scaled
    lb_v = din("lb_v", (RANK, C), dt=BF16)   # v-lora B, scaled
    wproj = din("wproj", (128, NCH, C), dt=BF16)  # proj-lora folded in
    masks = din("masks", (128, NMASK, 128), dt=BF16)
    yout = nc.dram_tensor("yout", [T, C], BF16, kind="ExternalOutput").ap()

    with TileContext(nc) as tc:
        ctx = contextlib.ExitStack()
        with ctx:
            ctx.enter_context(
                nc.allow_low_precision(reason="float32r is full-width fp32 storage")
            )
            # --- persistent SBUF ---
            persist = ctx.enter_context(tc.tile_pool(name="persist", bufs=1))
            ktsb = persist.tile([128, NCH, SEQ], BF16)      # resident K^T
            vres = persist.tile([128, 16, H, D + 1], BF16)  # resident V + ones col
            qT_sb = persist.tile([128, NCH, T], BF16)       # resident Q^T (prescaled)
            y_acc = persist.tile([128, NCH, T], BF16)       # normalized y
            tmp_kv = persist.tile([RANK, T], BF16)          # e2 attn-lora mid

            nc.vector.memset(vres[:, :, :, D : D + 1], 1.0)  # ones column

            # --- PSUM: 3x2 (score duos / unit psum) + 2x1 (q-major AV)
            ps_s = ctx.enter_context(tc.tile_pool(name="ps_s", bufs=3, space="PSUM"))
            ps_av = ctx.enter_context(tc.tile_pool(name="ps_av", bufs=2, space="PSUM"))

            pt_pool = ctx.enter_context(tc.tile_pool(name="pt", bufs=12))
            ynorm_pool = ctx.enter_context(tc.tile_pool(name="ynorm", bufs=3))

            holders = {}

            def _norm_stage(dqb, dp, yq):
                """Normalize q-major AV output and XBAR-transpose into y_acc.

                yq[hi] is [128 q, 4 qt, D+1] PSUM (col D = denominator).  The
                division is a per-partition scalar mul; the [q, (hi d)] ->
                [(hi d), q] transpose rides the DMA xbar (16x128 tiles), so
                the PE never touches normalization.
                """
                y_acc = holders["y_acc"]
                y_nm = ynorm_pool.tile([128, 4, 2, D], BF16, tag="yn")
                r_ts = []
                for hi in range(2):
                    r_t = ynorm_pool.tile([128, 4, 1], F32, tag=f"r{hi}")
                    nc.vector.reciprocal(r_t[:], yq[hi][:, :, D : D + 1])
                    r_ts.append(r_t)
                for qt in range(4):
                    for hi in range(2):
                        nc.vector.tensor_scalar_mul(
                            y_nm[:, qt, hi, :],
                            yq[hi][:, qt, 0:D],
                            r_ts[hi][:, qt, 0:1],
                        )
                qg = 512 * dqb
                nc.sync.dma_start_transpose(
                    out=y_acc[:, dp, qg : qg + 512].rearrange(
                        "p (b c) -> p b c", b=4
                    ),
                    in_=y_nm[:],
                )

            def emit_attention_p(qb, p, bracket=(), pre_norm=()):
                """Attention for q-block qb, head pair (2p, 2p+1).

                Scores land k-major ([128 k, nq]); AV uses the probability
                tile as the 128-wide stationary with the 65-col V tile
                moving.  AV for tile ki is emitted after tile ki+1's scores
                so the exp latency is hidden; bracket thunks (deferred
                K/V/proj units) are popped one per k-tile to fill the
                exp-bound stretches.
                """
                items = list(bracket)
                ktl = _ktiles_for_qblock(qb)
                if (qb, p) == (0, 0):
                    # p0 runs while x s2/s3 + masks are still streaming: do
                    # the s0/s1-backed e1 tiles first and slot the
                    # x2-dependent K/V units in via the bracket.
                    ktl = [t for t in ktl if t[0] == "e1"] + [
                        t for t in ktl if t[0] == "e2"
                    ]
                else:
                    ktl = sorted(
                        ktl, key=lambda rj: -len(_active_qts(rj[0], rj[1], qb))
                    )
                first_ki, last_ki = {}, {}
                for ki, (region, j) in enumerate(ktl):
                    for qt in _active_qts(region, j, qb):
                        first_ki.setdefault(qt, ki)
                        last_ki[qt] = ki
                yq = [
                    ps_av.tile([128, 4, D + 1], F32, tag="av",
                               name=f"yq_{qb}_{p}_{i}")
                    for i in range(2)
                ]
                # start marks the whole 2KB psum bank pending-zero, so it may
                # appear exactly once per bank (first AV matmul of the pair);
                # per-byte pending-zero then gives zero-then-accumulate per
                # qt region.  stop closes the bank group on the last matmul.
                av_total = sum(len(_active_qts(r_, j_, qb)) for r_, j_ in ktl)
                av_count = [0, 0]
                avq = []

                def flush_av():
                    pt0, coff0, qts0, qlo0, st0 = avq.pop(0)
                    for hi in range(2):
                        for qt in qts0:
                            rel = slice(coff0 + 128 * (qt - qlo0),
                                        coff0 + 128 * (qt - qlo0 + 1))
                            nc.tensor.matmul(
                                yq[hi][:, qt - 4 * qb, :],
                                pt0[:, hi, rel],
                                vres[:, st0, 2 * p + hi, :],
                                start=(av_count[hi] == 0),
                                stop=(av_count[hi] == av_total - 1),
                                skip_group_check=True,
                            )
                            av_count[hi] += 1

                # pack adjacent tiles into <=512-col groups: one psum tile,
                # ONE exp per group (saves the ~185ns activation init per
                # merged tile, right in the exp-bound stretches).  Within a
                # shared bank only the group's first score matmul carries
                # start (whole-bank pending-zero) and the last carries stop.
                groups, cur, w = [], [], 0
                for t in ktl:
                    qw_t = len(_active_qts(t[0], t[1], qb))
                    if cur and w + qw_t > 4:
                        groups.append(cur)
                        cur, w = [], 0
                    cur.append(t)
                    w += qw_t
                if cur:
                    groups.append(cur)

                for gi, grp in enumerate(groups):
                    if items and gi >= 1:
                        items.pop(0)()

                    sp = ps_s.tile([128, 2, 512], F32, tag="s")
                    pt = pt_pool.tile([128, 2, 512], BF16, tag="pt")
                    coff = 0
                    entries = []
                    for ti, (region, j) in enumerate(grp):
                        qts = _active_qts(region, j, qb)
                        qlo, qw = qts[0], len(qts)
                        q_sl = slice(128 * qlo, 128 * (qlo + qw))
                        nq = 128 * qw
                        kbase = (0 if region == "e1" else T) + 128 * j
                        for hi in range(2):
                            lo = 64 * hi
                            nc.tensor.matmul(
                                sp[:, hi, coff : coff + nq],
                                ktsb[lo : lo + 64, p, kbase : kbase + 128],
                                qT_sb[lo : lo + 64, p, q_sl],
                                start=(ti == 0),
                                stop=(ti == len(grp) - 1),
                                skip_group_check=True,
                            )
                        entries.append((coff, qts, qlo, kbase // 128,
                                        region, j))
                        coff += nq
                    nc.scalar.activation(
                        pt[:, :, 0:coff],
                        sp[:, :, 0:coff],
                        mybir.ActivationFunctionType.Exp,
                    )
                    for coff0, qts, qlo, st_glob, region, j in entries:
                        for qt in qts:
                            if (region, j, qt) in MASK_IDX:
                                mi = MASK_IDX[(region, j, qt)]
                                rel = slice(coff0 + 128 * (qt - qlo),
                                            coff0 + 128 * (qt - qlo + 1))
                                mb = mask_sb[:, mi : mi + 1, :].broadcast_to(
                                    [128, 2, 128]
                                )
                                nc.vector.tensor_mul(
                                    pt[:, :, rel], pt[:, :, rel], mb
                                )
                        avq.append((pt, coff0, qts, qlo, st_glob))
                while avq:
                    flush_av()
                for it in items:
                    it()
                for it in pre_norm:
                    it()
                _norm_stage(qb, p, yq)

            # ===== phase A: projections (compensated fp8 DoubleRow) ===========
            def mid_group(out_ap, x8_t, xlo_t, sl):
                """attn-lora mid: sum_c A[c, :]^T x[c, sl] -> [RPAD, 512]."""
                i = 0
                for lh, rh in (
                    (la_sb[:, 0], x8_t), (la_sb[:, 1], x8_t), (la_sb[:, 0], xlo_t),
                ):
                    for cp in range(NCP):
                        nc.tensor.matmul(
                            out_ap,
                            lh[:, cp, :, :],
                            rh[:, cp, :, sl],
                            start=(i == 0),
                            stop=(i == 3 * NCP - 1),
                            perf_mode=DR,
                        )
                        i += 1

            if True:
                xh = {}

                def xap(s):
                    if s < 3:
                        t = xh["xe"]
                        return t[:, 0], t[:, 1], 512 * s
                    t = xh["x3"]
                    return t[:, 0], t[:, 1], 0

                def load_x(s):
                    sl = slice(512 * s, 512 * (s + 1))
                    off = 512 * s if s < 3 else 0
                    dst = xh["xe"] if s < 3 else xh["x3"]
                    nc.sync.dma_start(
                        out=dst[:, :, :, :, off : off + 512],
                        in_=x_p[:, :, :, :, sl],
                    )

                # ---- Q^T projection first (own T rows), resident.
                # Q-side LoRA is folded into wq on the host. ----
                # early-needed weights take fresh SBUF before the Q pools
                # (allocating them after the Q pools close would place them
                # in the freed zones, stalling their DMAs behind the last Q
                # matmul).  LIFO: wk/wva/xres outlive qtmp.
                wk_pool = ctx.enter_context(tc.tile_pool(name="wk", bufs=1))
                wk_sb = wk_pool.tile([128, 2, NCP, 2, C], F8E4)
                wva_pool = ctx.enter_context(tc.tile_pool(name="wva", bufs=1))
                wva_sb = wva_pool.tile([128, 2, NCP, 2, 512], F8E4)
                xres = ctx.enter_context(tc.tile_pool(name="xres", bufs=1))
                xh["xe"] = xres.tile([128, 2, NCP, 2, 3 * 512], F8E4, name="xe",
                                     tag="xe")
                with tc.tile_pool(name="qtmp", bufs=1) as xa_pool:
                    if True:
                        xq_sb = xa_pool.tile([128, 2, 2, NCP, 2, 512], F8E4)
                        wq_sb = xa_pool.tile([128, 8, 2, NCP, 2, 128], F8E4)
                        nc.sync.dma_start(
                            out=xq_sb[:, 0, 0, 0:2], in_=xq_p[0, 0][:, 0:2]
                        )
                        nc.sync.dma_start(out=wq_sb[:, 0], in_=wq_p[0])
                        nc.sync.dma_start(
                            out=xq_sb[:, 0, 0, 2:4], in_=xq_p[0, 0][:, 2:4]
                        )
                        nc.sync.dma_start(out=wq_sb[:, 1], in_=wq_p[1])
                        nc.sync.dma_start(out=xq_sb[:, 0, 1], in_=xq_p[0, 1])
                        for m_ in range(2, 8):
                            nc.sync.dma_start(out=wq_sb[:, m_], in_=wq_p[m_])
                        nc.sync.dma_start(
                            out=xq_sb[:, 1, 0, 0:2], in_=xq_p[1, 0][:, 0:2]
                        )
                        nc.sync.dma_start(
                            out=xq_sb[:, 1, 0, 2:4], in_=xq_p[1, 0][:, 2:4]
                        )
                        nc.sync.dma_start(out=xq_sb[:, 1, 1], in_=xq_p[1, 1])
                        nc.sync.dma_start(out=wk_sb[:], in_=wk_p[:])
                        load_x(0)
                        load_x(1)
                        nc.sync.dma_start(out=wva_sb[:], in_=wva_p[:])
                        load_x(2)

                        for s_ in range(2):
                            sl = slice(s_ * 512, (s_ + 1) * 512)
                            for mp in range(4):
                                qps = ps_s.tile([128, 2, 512], F32, tag="s")
                                for h2 in range(2):
                                    m = 2 * mp + h2
                                    i = 0
                                    for wl, xl in ((0, 0), (1, 0), (0, 1)):
                                        for cp in range(NCP):
                                            nc.tensor.matmul(
                                                qps[:, h2, :],
                                                wq_sb[:, m, wl, cp, :, :],
                                                xq_sb[:, s_, xl, cp, :, :],
                                                start=(i == 0),
                                                stop=(i == 3 * NCP - 1),
                                                perf_mode=DR,
                                            )
                                            i += 1
                                nc.scalar.mul(
                                    qT_sb[:, 2 * mp : 2 * mp + 2, sl], qps[:],
                                    1.0 / WSC,
                                )

                # late-needed tensors go to the zones freed by the Q pools
                # (their DMAs legitimately wait out the last Q matmul).
                late_pool = ctx.enter_context(tc.tile_pool(name="late", bufs=1))
                mask_sb = late_pool.tile([128, NMASK, 128], BF16)
                la_sb = late_pool.tile([128, 2, NCP, 2, RPAD], F8E4)
                lb_k_sb = late_pool.tile([RANK, C], BF16)
                lb_v_sb = late_pool.tile([RANK, C], BF16)
                wvb_pool = ctx.enter_context(tc.tile_pool(name="wvb", bufs=1))
                wvb_sb = wvb_pool.tile([128, 2, NCP, 2, 512], F8E4)
                x3res = ctx.enter_context(tc.tile_pool(name="x3", bufs=1))
                xh["x3"] = x3res.tile([128, 2, NCP, 2, 512], F8E4, name="x3")

                nc.sync.dma_start(out=mask_sb[:], in_=masks[:])
                nc.sync.dma_start(out=la_sb[:], in_=la_p[:])
                nc.sync.dma_start(out=lb_k_sb[:], in_=lb_k[:])
                nc.sync.dma_start(out=lb_v_sb[:], in_=lb_v[:])
                load_x(3)
                nc.sync.dma_start(out=wvb_sb[:], in_=wvb_p[:])

                holders["y_acc"] = y_acc
                stage_rr = [0]

                def _stage(dst, src):
                    """Window staging alternates DVE/Act; during qb1 the Act
                    engine is exp-saturated so staging stays on DVE."""
                    stage_rr[0] += 1
                    if holders.get("dve_only") or stage_rr[0] % 2 == 0:
                        nc.vector.tensor_scalar_mul(dst, src, 1.0 / WSC)
                    else:
                        nc.scalar.mul(dst, src, 1.0 / WSC)

                def emit_mid(s):
                    tsl = slice((s - 2) * 512, (s - 1) * 512)
                    x8_t, xlo_t, off = xap(s)
                    sl = slice(off, off + 512)
                    tmp_ps = ps_s.tile([128, 2, 512], F32, tag="s")
                    mid_group(tmp_ps[0:RPAD, 0, :], x8_t, xlo_t, sl)
                    nc.vector.tensor_scalar_mul(
                        tmp_kv[:, tsl], tmp_ps[0:RANK, 0, :], 1.0 / WSC
                    )

                def emit_k_duo(s, mp, window=False):
                    sl = slice(s * 512, (s + 1) * 512)
                    tsl = slice((s - 2) * 512, (s - 1) * 512) if s >= 2 else None
                    x8_t, xlo_t, off = xap(s)
                    xsl = slice(off, off + 512)
                    kps = ps_s.tile([128, 2, 512], F32, tag="s")
                    for h2 in range(2):
                        m = 2 * mp + h2
                        cols = slice(128 * m, 128 * (m + 1))
                        i = 0
                        for lh, rh in (
                            (wk_sb[:, 0], x8_t), (wk_sb[:, 1], x8_t),
                            (wk_sb[:, 0], xlo_t),
                        ):
                            for cp in range(NCP):
                                nc.tensor.matmul(
                                    kps[:, h2, :],
                                    lh[:, cp, :, cols],
                                    rh[:, cp, :, xsl],
                                    start=(i == 0),
                                    stop=(i == 3 * NCP - 1 and s < 2),
                                    perf_mode=DR,
                                )
                                i += 1
                        if s >= 2:
                            nc.tensor.matmul(
                                kps[:, h2, :],
                                lb_k_sb[:, 128 * m : 128 * (m + 1)],
                                tmp_kv[:, tsl],
                                start=False,
                                stop=True,
                            )
                    dst = ktsb[:, 2 * mp : 2 * mp + 2, sl]
                    if window:
                        _stage(dst, kps[:])
                    else:
                        nc.scalar.mul(dst, kps[:], 1.0 / WSC)

                def emit_v_single(s, st, vc, window=False):
                    x8_t, xlo_t, off = xap(s)
                    ssl = slice(off + 128 * st, off + 128 * (st + 1))
                    wvh = wva_sb if vc == 0 else wvb_sb
                    vps = ps_s.tile([128, 2, 512], F32, tag="s")
                    i = 0
                    for lh, rh in (
                        (x8_t, wvh[:, 0]), (xlo_t, wvh[:, 0]),
                        (x8_t, wvh[:, 1]),
                    ):
                        for cp in range(NCP):
                            nc.tensor.matmul(
                                vps[:, 0, :],
                                lh[:, cp, :, ssl],
                                rh[:, cp, :, :],
                                start=(i == 0),
                                stop=(i == 3 * NCP - 1 and s < 2),
                                perf_mode=DR,
                            )
                            i += 1
                    if s >= 2:
                        base = (s - 2) * 512 + 128 * st
                        nc.tensor.matmul(
                            vps[:, 0, :],
                            tmp_kv[:, base : base + 128],
                            lb_v_sb[:, 512 * vc : 512 * vc + 512],
                            start=False,
                            stop=True,
                        )
                    dst = vres[:, 4 * s + st, 8 * vc : 8 * vc + 8, 0:D]
                    vsrc = vps[:, 0, :].rearrange("p (h d) -> p h d", h=8)
                    if window:
                        _stage(dst, vsrc)
                    else:
                        nc.scalar.mul(dst, vsrc, 1.0 / WSC)

                # ---- phase A: only what q-block-0 pair 0 needs up front:
                # mids, K chunk-pair 0, V head-half 0.  Everything else is
                # deferred into attention windows ordered by first consumer.
                emit_k_duo(0, 0)
                emit_k_duo(1, 0)
                for s_ in range(2):
                    for st in range(4):
                        emit_v_single(s_, st, 0)
                def K(s, mp):
                    return lambda: emit_k_duo(s, mp, window=True)

                def V(s, st, vc):
                    return lambda: emit_v_single(s, st, vc, window=True)

                def MID(s):
                    return lambda: emit_mid(s)

                def K20():
                    emit_mid(2)
                    emit_k_duo(2, 0)

                brackets0 = {
                    0: [K20, lambda: emit_v_single(2, 0, 0),
                        lambda: emit_v_single(2, 1, 0),
                        lambda: emit_v_single(2, 2, 0),
                        lambda: emit_v_single(2, 3, 0), MID(3)],
                    1: [K(0, 1), K(1, 1), K(2, 1), V(0, 0, 1), V(0, 1, 1)],
                    2: [V(0, 2, 1), V(0, 3, 1), V(1, 0, 1), K(0, 2), K(1, 2)],
                    3: [K(2, 2), V(1, 1, 1), V(1, 2, 1), V(1, 3, 1),
                        V(2, 0, 1)],
                    4: [V(2, 1, 1), V(2, 2, 1), V(2, 3, 1), K(0, 3), K(1, 3),
                        K(2, 3)],
                    5: [K(3, 0), V(3, 0, 0)],
                    6: [V(3, 1, 0), V(3, 2, 0)],
                    7: [V(3, 3, 0)],
                }
                for p in range(5):
                    emit_attention_p(0, p, bracket=brackets0.get(p, ()))
                # x s0-2 is dead after p4's windows: wproj takes over the
                # same slot (tag reuse adds the WAR dependency on the last
                # x-reader automatically).
                wproj_sb = xres.tile([128, NCH, C], BF16, name="wproj_sb",
                                     tag="xe")
                nc.sync.dma_start(out=wproj_sb[:], in_=wproj[:])
                for p in range(5, 8):
                    emit_attention_p(0, p, bracket=brackets0.get(p, ()))

            # ===== phase B: qb1 attention + output projection =================
            ost_pool = ctx.enter_context(tc.tile_pool(name="ost", bufs=2))

            def proj_qs(qb, qs):
                qrow = 512 * qb + 128 * qs
                ops = ps_s.tile([128, 2, 512], F32, tag="s")
                for co in range(2):
                    cos = slice(512 * co, 512 * (co + 1))
                    for ch in range(NCH):
                        nc.tensor.matmul(
                            ops[:, co, :],
                            y_acc[:, ch, qrow : qrow + 128],
                            wproj_sb[:, ch, cos],
                            start=(ch == 0),
                            stop=(ch == NCH - 1),
                        )
                ost = ost_pool.tile([128, 2, 512], BF16, tag="ost")
                for co in range(2):
                    if qb == 1 and co == 1:
                        nc.scalar.copy(ost[:, co, :], ops[:, co, :])
                    else:
                        nc.vector.tensor_copy(ost[:, co, :], ops[:, co, :])
                    nc.sync.dma_start(
                        out=yout[qrow : qrow + 128, 512 * co : 512 * co + 512],
                        in_=ost[:, co, :],
                    )

            holders["dve_only"] = True
            brackets1 = {
                0: [K(3, 1)],
                1: [lambda: proj_qs(0, 0), V(3, 0, 1)],
                2: [K(3, 2), lambda: proj_qs(0, 1)],
                3: [V(3, 1, 1), V(3, 2, 1), V(3, 3, 1)],
                4: [K(3, 3), lambda: proj_qs(0, 2)],
                5: [lambda: proj_qs(0, 3)],
            }
            hold = {}

            def proj_partial(qs):
                # chunks 0..6 don't need pair 7's normalize; run them in the
                # dead window between p7's AV drain and its norm chain.
                qrow = 512 + 128 * qs
                ops = ps_s.tile([128, 2, 512], F32, tag="s",
                                name=f"opspart_{qs}")
                for co in range(2):
                    cos = slice(512 * co, 512 * (co + 1))
                    for ch in range(NCH - 1):
                        nc.tensor.matmul(
                            ops[:, co, :],
                            y_acc[:, ch, qrow : qrow + 128],
                            wproj_sb[:, ch, cos],
                            start=(ch == 0),
                            stop=False,
                        )
                hold[qs] = ops

            def proj_finish(qs):
                ops = hold.pop(qs)
                qrow = 512 + 128 * qs
                for co in range(2):
                    cos = slice(512 * co, 512 * (co + 1))
                    nc.tensor.matmul(
                        ops[:, co, :],
                        y_acc[:, NCH - 1, qrow : qrow + 128],
                        wproj_sb[:, NCH - 1, cos],
                        start=False,
                        stop=True,
                    )
                ost = ost_pool.tile([128, 2, 512], BF16, tag="ost")
                for co in range(2):
                    if co == 1:
                        nc.scalar.copy(ost[:, co, :], ops[:, co, :])
                    else:
                        nc.vector.tensor_copy(ost[:, co, :], ops[:, co, :])
                    nc.sync.dma_start(
                        out=yout[qrow : qrow + 128, 512 * co : 512 * co + 512],
                        in_=ost[:, co, :],
                    )

            for p in range(8):
                emit_attention_p(
                    1, p, bracket=brackets1.get(p, ()),
                    pre_norm=(
                        [lambda: proj_partial(0), lambda: proj_partial(1)]
                        if p == 7 else ()
                    ),
                )

            proj_finish(0)
            proj_finish(1)
            for qs in range(2, 4):
                proj_qs(1, qs)
    return nc


_PROGRAM = None


def _get_program():
    global _PROGRAM
    if _PROGRAM is None:
        _PROGRAM = _build_program()
    return _PROGRAM


# ---------------------------------------------------------------------------
# Host side
# ---------------------------------------------------------------------------
def _delayed_mask_np(t):
    ones = np.ones((t, t), dtype=bool)
    m11 = np.tril(ones) & np.triu(ones, -(LOOKAHEAD + OVERLAP))
    m12 = np.tril(ones, -LOOKAHEAD)
    m21 = np.tril(ones, LOOKAHEAD) & np.triu(ones, -OVERLAP)
    m22 = np.tril(ones)
    return np.block([[m11, m12], [m21, m22]])


def _fp8_pair(a):
    hi = a.astype(FP8NP)
    lo = (a - hi.astype(np.float32)).astype(FP8NP)
    return hi, lo


def _cp_layout(m):
    """[C, N] -> [128, NCP, 2, N] with c = 256*cp + 128*i + p."""
    n = m.shape[1]
    return np.ascontiguousarray(m.reshape(NCP, 2, 128, n).transpose(2, 0, 1, 3))


def _pack_pair(m):
    """[C, N] f32 -> [128, 2, NCP, 2, N] fp8 (hi/lo on axis 1)."""
    hi, lo = _fp8_pair(_cp_layout(m))
    return np.ascontiguousarray(np.stack([hi, lo], axis=1))


def kernel(
    e1,
    e2,
    W_attn,
    W_proj,
    lora_A_attn,
    lora_B_attn,
    lora_A_proj,
    lora_B_proj,
    _trace=False,
):
    f32 = np.float32
    bf16 = ml_dtypes.bfloat16
    e1 = np.asarray(e1, f32)
    e2 = np.asarray(e2, f32)
    W_attn = np.asarray(W_attn, f32)
    W_proj = np.asarray(W_proj, f32)
    lora_A_attn = np.asarray(lora_A_attn, f32)
    lora_B_attn = np.asarray(lora_B_attn, f32)
    lora_A_proj = np.asarray(lora_A_proj, f32)
    lora_B_proj = np.asarray(lora_B_proj, f32)
    nc = _get_program()
    M = _delayed_mask_np(T)

    # --- role-independent prep (once) ---
    wk_pk = _pack_pair(W_attn[:, C : 2 * C] * WSC)
    wv_full = _pack_pair(W_attn[:, 2 * C :] * WSC)
    wva_pk = np.ascontiguousarray(wv_full[..., 0:512])
    wvb_pk = np.ascontiguousarray(wv_full[..., 512:])
    la_pad = np.zeros((C, RPAD), f32)
    la_pad[:, :RANK] = lora_A_attn * WSC
    la_pk = _pack_pair(la_pad)
    lb_k = np.ascontiguousarray(
        lora_B_attn[:, C : 2 * C] * (LSCALE * WSC)
    ).astype(bf16)
    lb_v = (np.ascontiguousarray(lora_B_attn[:, 2 * C :]) * (LSCALE * WSC)).astype(
        bf16
    )
    lora_dq = LSCALE * (lora_A_attn @ lora_B_attn[:, :C])   # e2-role q delta
    lora_dp = LSCALE * (lora_A_proj @ lora_B_proj)          # e2-role proj delta

    wqm_r, wproj_r, masks_r = {}, {}, {}
    for r in (0, 1):
        wq_eff = (W_attn[:, :C] + r * lora_dq) * (WSC * QSCALE)
        full = _pack_pair(wq_eff)  # [128, 2, NCP, 2, C]
        # column-chunk-major: [8, 128, 2, NCP, 2, 128]
        wqm_r[r] = np.ascontiguousarray(
            full.reshape(128, 2, NCP, 2, 8, 128).transpose(4, 0, 1, 2, 3, 5)
        )
        wproj_r[r] = np.ascontiguousarray(
            (W_proj + r * lora_dp).reshape(NCH, 128, C).transpose(1, 0, 2)
        ).astype(bf16)
        mk = np.empty((128, NMASK, 128), dtype=bf16)
        for i, (region, j, qt) in enumerate(MASK_TILES):
            qg = r * T + 128 * qt
            kg = (0 if region == "e1" else T) + 128 * j
            mk[:, i, :] = M[qg : qg + 128, kg : kg + 128].T.astype(f32)
        masks_r[r] = mk

    in_maps = []
    x_cache = None
    for core in range(8):
        b, r = core // 2, core % 2
        if r == 0:
            x = np.concatenate([e1[b], e2[b]], axis=0)  # [2T, C]
            x_cache = _pack_pair(np.ascontiguousarray(x.T))
        xpk = x_cache
        qsl0 = slice(r * T, r * T + 512)
        qsl1 = slice(r * T + 512, (r + 1) * T)
        xq_pk = np.ascontiguousarray(np.stack(
            [xpk[..., qsl0], xpk[..., qsl1]], axis=0
        ).transpose(0, 2, 1, 3, 4, 5))  # [2s, 2hl, 128, NCP, 2, 512]
        in_maps.append({
            "x_p": xpk,
            "xq_p": xq_pk,
            "wq_p": wqm_r[r],
            "wk_p": wk_pk,
            "wva_p": wva_pk,
            "wvb_p": wvb_pk,
            "la_p": la_pk,
            "lb_k": lb_k,
            "lb_v": lb_v,
            "wproj": wproj_r[r],
            "masks": masks_r[r],
        })

    res = run_bass_kernel_spmd(nc, in_maps, core_ids=list(range(8)), trace=_trace)
    y1 = np.stack(
        [res.results[2 * b]["yout"].astype(np.float32) for b in range(B)]
    )
    y2 = np.stack(
        [res.results[2 * b + 1]["yout"].astype(np.float32) for b in range(B)]
    )
    if _trace:
        kernel.last_results = res
    return y1, y2
